# revision 39
# baseline (speedup 1.0000x reference)
"""GCN aggregator kernel for Trainium2 (8 NeuronCores, batch-sharded).

Math (faithful to the reference):
    mask[n, c] = 1 iff c in set(neigh_idx[n, :]) | {nodes[n]}     (N x M 0/1)
    out = diag(1/sqrt(row_sum)) @ mask @ diag(1/sqrt(max(col_sum,1))) @ E

Per-core (512 rows) device algorithm:
  1. Load idx slab [512, 33] as [128p, 4nb, 33k] plus the same entries as an
     int16 "wrapped" list (entry i at partition i%16, col i//16, replicated
     across the eight 16-partition groups) for the gpsimd dma_gather units.
     Entry order: i = g*128 + p with g = nb*33 + k, so gathered row i lands
     at [partition p, slot g] matching the [p, nb, k] index tile.
  2. Main gather G[p, g, :] = E[idx, :] via chunked dma_gather (<=1024
     indices per op - larger single ops overflow the SWDGE ring).
  3. Dedup: prefix duplicate count per row -> w in {0,1} (first-occurrence
     flag); row_cnt = sum_k w; duplicates get idx+16384 so their one-hot
     never fires (hi >= 128).
  4. Histogram: hi/lo split (c = 128*hi + lo); one-hots L[i,hi], R[i,lo] in
     bf16 (exact 0/1); count[hi,lo] += L_c.T @ R_c over 132 chunks of 128
     entries (PSUM f32 accumulate - exact integer arithmetic).
  5. AllReduce the [128, 128] partial count across the 8 cores.
  6. Per-entry count: dma_gather 64-float rows of the reduced table at
     idx>>6, then select column idx&63 with a bf16 one-hot dot (exact).
  7. cv = 1/sqrt(max(cnt,1)); alpha = w * cv / sqrt(row_cnt);
     out[n, :] = sum_k alpha[n, k] * G[n, k, :]  (DVE mul + reduce).

The tail (6..7) is pipelined over the four 128-row blocks.
"""

import numpy as np

N, K, M, D = 4096, 32, 16384, 128
NCORES = 8
NPR = N // NCORES  # 512 rows per core
KP1 = K + 1  # 33 entries per row
P = 128
NB = NPR // P  # 4 row-blocks per core
GW = NB * KP1  # 132 entries per partition
NI = P * GW  # 16896 entries per core
GCH = 1024  # dma_gather chunk (hardware limit ~1024-1535 idx/op)

_NC_CACHE = {}


def _apply_tile_patches():
    """Work around this walrus build's 1-embedded-sync-wait-per-instruction
    limit: split the kernel-tail drain (the one place Tile emits a
    multi-wait instruction unconditionally) into a chain of single-wait
    drains. SP is in-order, so this is equivalent."""
    import concourse.mybir as mybir
    import concourse.tile as tile
    import concourse.tile_sem_assignment as tsa

    # Cap the DMA completion-sem lanes so the drain chain stays short.
    tsa.NUM_SWDGE_GLOBAL_SEMS = 2

    if getattr(tile.TileContext, "_split_drain_patch", False):
        return
    from concourse.vector_clock import ScopedClock

    def _drain_and_barrier(self, tick_clock, wait_clock):
        probe = self.nc.sync.drain()
        wait_clock.add_sem_waits(
            probe.ins, ScopedClock({None: tick_clock.global_clock})
        )
        si = probe.ins.sync_info
        waits = list(si.on_wait) if si is not None else []
        if len(waits) > 1:
            si.on_wait = waits[:1]
            for w in waits[1:]:
                d = self.nc.sync.drain()
                dsi = d.ins.sync_info
                if dsi is None:
                    d.ins.sync_info = mybir.SyncInfo(on_wait=[w], on_update=[])
                else:
                    dsi.on_wait = [w]
        self.nc.all_engine_barrier()
        assert self.sems is not None
        popped = self.nc._tile_sem_poison_stack.pop()
        assert popped is self._sem_poison
        self.nc.clear_and_free_semaphores(list(self.sems.allocated().values()))
        self.nc.all_engine_barrier()

    tile.TileContext._drain_and_barrier = _drain_and_barrier
    tile.TileContext._split_drain_patch = True


def _chunked_gather(nc, out_view, src_ap, idx_tile, elem):
    """Issue dma_gather in <=GCH-index chunks. out_view: [128, GW, elem],
    idx_tile: int16 [128, NI//16] wrapped."""
    pos = 0
    while pos < NI:
        ch = min(GCH, NI - pos)
        nc.gpsimd.dma_gather(
            out_view[:, pos // P : (pos + ch) // P, :],
            src_ap,
            idx_tile[:, pos // 16 : (pos + ch) // 16],
            ch,
            ch,
            elem,
        )
        pos += ch


def _build_nc(reps=1, ablate=()):
    import concourse.bacc as bacc
    import concourse.mybir as mybir
    import concourse.tile as tile
    from contextlib import ExitStack

    _apply_tile_patches()

    dt = mybir.dt
    Alu = mybir.AluOpType
    Act = mybir.ActivationFunctionType

    nc = bacc.Bacc(
        "TRN2",
        target_bir_lowering=False,
        debug=False,
        num_devices=NCORES,
    )

    idx_d = nc.dram_tensor("idx", [NPR, KP1], dt.int32, kind="ExternalInput")
    idxw_d = nc.dram_tensor("idxw", [P, NI // 16], dt.int16, kind="ExternalInput")
    emb_d = nc.dram_tensor("embed", [M, D], dt.float32, kind="ExternalInput")
    out_d = nc.dram_tensor("out", [NPR, D], dt.float32, kind="ExternalOutput")

    with tile.TileContext(nc) as tc, ExitStack() as ctx:
        sb = ctx.enter_context(tc.tile_pool(name="sb", bufs=1))
        ps = ctx.enter_context(tc.tile_pool(name="ps", bufs=1, space="PSUM"))
        dr = ctx.enter_context(tc.tile_pool(name="dr", bufs=1, space="DRAM"))
        sb2 = ctx.enter_context(tc.tile_pool(name="sb2", bufs=2))

        def _body():
          # ---- load wrapped idx (for gathers) and [p, nb, k] idx (for compute)
         IW = sb.tile([P, NI // 16], dt.int16)
         nc.gpsimd.dma_start(out=IW[:], in_=idxw_d.ap())
         I32 = sb.tile([P, NB, KP1], dt.int32)
         nc.gpsimd.dma_start(
             out=I32[:], in_=idx_d.ap().rearrange("(nb p) k -> p nb k", p=P)
         )

         # ---- main gather (starts immediately, overlaps everything below)
         G = sb.tile([P, GW, D], dt.float32)
         if "nogather" in ablate:
             nc.vector.memset(G[:, 0:1, :], 1.0)
         else:
             _chunked_gather(nc, G[:], emb_d.ap(), IW[:], D)

         # ---- int16 indices for cheap exact compares
         I16 = sb.tile([P, NB, KP1], dt.int16)
         nc.vector.tensor_copy(out=I16[:], in_=I32[:])

         # ---- prefix duplicate count: acc[p,nb,k] = #{j<k : idx_j == idx_k}
         acc = sb.tile([P, NB, KP1], dt.int16)
         tmp = sb.tile([P, NB, KP1], dt.int16)
         nc.vector.memset(acc[:], 0)
         for j in ([] if "nodedup" in ablate else range(KP1 - 1)):
             rest = KP1 - 1 - j
             nc.vector.tensor_tensor(
                 out=tmp[:, :, j + 1 :],
                 in0=I16[:, :, j + 1 :],
                 in1=I16[:, :, j : j + 1].to_broadcast([P, NB, rest]),
                 op=Alu.is_equal,
             )
             nc.vector.tensor_tensor(
                 out=acc[:, :, j + 1 :],
                 in0=acc[:, :, j + 1 :],
                 in1=tmp[:, :, j + 1 :],
                 op=Alu.add,
             )

         # ---- first-occurrence flag w and row counts
         w16 = sb.tile([P, NB, KP1], dt.int16)
         nc.vector.tensor_scalar(
             out=w16[:], in0=acc[:], scalar1=0, scalar2=None, op0=Alu.is_equal
         )
         wf = sb.tile([P, NB, KP1], dt.float32)
         nc.vector.tensor_copy(out=wf[:], in_=w16[:])
         rowcnt = sb.tile([P, NB], dt.float32)
         nc.vector.tensor_reduce(
             out=rowcnt[:], in_=wf[:], axis=mybir.AxisListType.X, op=Alu.add
         )
         rowsq = sb.tile([P, NB], dt.float32)
         nc.scalar.activation(out=rowsq[:], in_=rowcnt[:], func=Act.Sqrt)
         rowinv = sb.tile([P, NB], dt.float32)
         nc.vector.reciprocal(out=rowinv[:], in_=rowsq[:])

         # ---- idx_mod = idx + 16384*(1 - w): duplicates get hi >= 128
         im = sb.tile([P, NB, KP1], dt.int16)
         nc.vector.scalar_tensor_tensor(
             out=im[:],
             in0=w16[:],
             scalar=-16384,
             in1=I16[:],
             op0=Alu.mult,
             op1=Alu.add,
         )
         nc.vector.tensor_scalar(
             out=im[:], in0=im[:], scalar1=16384, scalar2=None, op0=Alu.add
         )
         hi = sb.tile([P, NB, KP1], dt.int16)
         nc.vector.tensor_scalar(
             out=hi[:], in0=im[:], scalar1=7, scalar2=None, op0=Alu.logical_shift_right
         )
         lo = sb.tile([P, NB, KP1], dt.int16)
         nc.vector.tensor_scalar(
             out=lo[:], in0=im[:], scalar1=127, scalar2=None, op0=Alu.bitwise_and
         )

         # ---- iota rows (gpsimd) + DVE-side copy so downstream wide TT ops
         # carry a single embedded sync wait
         iot0 = sb.tile([P, P], dt.int16)
         nc.gpsimd.iota(iot0[:], pattern=[[1, P]], base=0, channel_multiplier=0)
         iot = sb.tile([P, P], dt.int16)
         nc.vector.tensor_copy(out=iot[:], in_=iot0[:])

         # ---- one-hots in bf16 (exact 0/1), chunked per row-block so the
         # histogram matmuls can start early
         L = sb.tile([P, GW, P], dt.bfloat16)
         R = sb.tile([P, GW, P], dt.bfloat16)
         iot_b = iot[:].unsqueeze(1).to_broadcast([P, KP1, P])
         for nb in ([] if "noonehot" in ablate or "nohist" in ablate else range(NB)):
             s = slice(nb * KP1, (nb + 1) * KP1)
             nc.vector.tensor_tensor(
                 out=L[:, s, :],
                 in0=hi[:, nb, :].unsqueeze(2).to_broadcast([P, KP1, P]),
                 in1=iot_b,
                 op=Alu.is_equal,
             )
             nc.vector.tensor_tensor(
                 out=R[:, s, :],
                 in0=lo[:, nb, :].unsqueeze(2).to_broadcast([P, KP1, P]),
                 in1=iot_b,
                 op=Alu.is_equal,
             )

         # ---- histogram: count[q, r] = sum_i L[i, q] * R[i, r]
         cps = ps.tile([P, P], dt.float32)
         for c in ([] if "nohist" in ablate else range(GW)):
             nc.tensor.matmul(
                 out=cps[:],
                 lhsT=L[:, c, :],
                 rhs=R[:, c, :],
                 start=(c == 0),
                 stop=(c == GW - 1),
             )
         cnt_sb = sb.tile([P, P], dt.float32)
         if "nohist" in ablate:
             nc.vector.memset(cnt_sb[:], 8.0)
         else:
             nc.vector.tensor_copy(out=cnt_sb[:], in_=cps[:])

         # ---- AllReduce partial counts across the 8 cores
         cc_in = dr.tile([P, P], dt.float32)
         cc_out = dr.tile([P, P], dt.float32)
         nc.gpsimd.dma_start(out=cc_in[:], in_=cnt_sb[:])
         if "nocoll" in ablate:
             nc.gpsimd.dma_start(out=cc_out[:], in_=cnt_sb[:])
         else:
             nc.gpsimd.collective_compute(
                 "AllReduce",
                 Alu.add,
                 replica_groups=[list(range(NCORES))],
                 ins=[cc_in[:].opt()],
                 outs=[cc_out[:].opt()],
             )

         # ---- wrapped idx>>6 for the count-row gather (layout-preserving)
         IW6 = sb.tile([P, NI // 16], dt.int16)
         nc.vector.tensor_scalar(
             out=IW6[:], in0=IW[:], scalar1=6, scalar2=None,
             op0=Alu.logical_shift_right,
         )
         # one-hot of idx&63 (bf16, exact) for the in-row select
         lo6 = sb.tile([P, NB, KP1], dt.int16)
         nc.vector.tensor_scalar(
             out=lo6[:], in0=I16[:], scalar1=63, scalar2=None, op0=Alu.bitwise_and
         )
         oh64 = sb.tile([P, GW, 64], dt.bfloat16)
         nc.vector.tensor_tensor(
             out=oh64[:],
             in0=lo6[:].rearrange("p nb k -> p (nb k)").unsqueeze(2)
             .to_broadcast([P, GW, 64]),
             in1=iot[:, 0:64].unsqueeze(1).to_broadcast([P, GW, 64]),
             op=Alu.is_equal,
         )

         # ---- per-entry count rows: tbl64 = cc_out viewed [256, 64]
         tbl64 = cc_out[:].rearrange("q r -> (q r)").rearrange("(a b) -> a b", b=64)

         # Post-collective tail, pipelined per 128-row block: count-row
         # gather (DMA) of block nb+1 overlaps select/alpha/mul/reduce (DVE)
         # of block nb.
         osb = sb.tile([P, NB, D], dt.float32)
         EPB = KP1 * P  # 4224 entries per block
         for nb in range(NB):
             s = slice(nb * KP1, (nb + 1) * KP1)
             CR = sb2.tile([P, KP1, 64], dt.float32, tag="crblk")
             if "nocnt" in ablate:
                 nc.vector.memset(CR[:, 0:1, :], 1.0)
             else:
                 base = nb * EPB
                 pos = 0
                 while pos < EPB:
                     ch = min(GCH, EPB - pos)
                     nc.gpsimd.dma_gather(
                         CR[:, pos // P : (pos + ch) // P, :],
                         tbl64,
                         IW6[:, (base + pos) // 16 : (base + pos + ch) // 16],
                         ch,
                         ch,
                         64,
                     )
                     pos += ch
             # select count: cnt_e = sum_t CR * oh64
             nc.vector.tensor_tensor(
                 out=CR[:], in0=CR[:], in1=oh64[:, s, :], op=Alu.mult
             )
             cnt_e = sb2.tile([P, KP1], dt.float32, tag="cntblk")
             nc.vector.tensor_reduce(
                 out=cnt_e[:], in_=CR[:], axis=mybir.AxisListType.X, op=Alu.add
             )
             nc.vector.tensor_scalar(
                 out=cnt_e[:], in0=cnt_e[:], scalar1=1.0, scalar2=None, op0=Alu.max
             )
             cv_sq = sb2.tile([P, KP1], dt.float32, tag="cvsblk")
             nc.scalar.activation(out=cv_sq[:], in_=cnt_e[:], func=Act.Sqrt)
             cv = sb2.tile([P, KP1], dt.float32, tag="cvblk")
             nc.vector.reciprocal(out=cv[:], in_=cv_sq[:])
             # alpha = w * cv * rowinv
             al = sb2.tile([P, KP1], dt.float32, tag="alblk")
             nc.vector.tensor_tensor(
                 out=al[:], in0=cv[:], in1=wf[:, nb, :], op=Alu.mult
             )
             nc.vector.tensor_tensor(
                 out=al[:],
                 in0=al[:],
                 in1=rowinv[:, nb : nb + 1].to_broadcast([P, KP1]),
                 op=Alu.mult,
             )
             if "notail" in ablate:
                 continue
             nc.vector.tensor_tensor(
                 out=G[:, s, :],
                 in0=G[:, s, :],
                 in1=al[:].unsqueeze(2).to_broadcast([P, KP1, D]),
                 op=Alu.mult,
             )
             nc.vector.tensor_reduce(
                 out=osb[:, nb, :],
                 in_=G[:, s, :].rearrange("p k d -> p d k"),
                 axis=mybir.AxisListType.X,
                 op=Alu.add,
             )
         if "notail" in ablate:
             nc.vector.memset(osb[:, 0:1, :], 0.0)

         # ---- store [128, 4, 128] -> [512, 128]
         nc.gpsimd.dma_start(
             out=out_d.ap().rearrange("(nb p) d -> p nb d", p=P), in_=osb[:]
         )


        # repeated body for differential wall-clock timing
        for _rep in range(reps):
            _body()

    nc.compile()
    return nc


def get_nc(reps=1, ablate=()):
    key = ("nc", reps, tuple(ablate))
    if key not in _NC_CACHE:
        _NC_CACHE[key] = _build_nc(reps, tuple(ablate))
    return _NC_CACHE[key]


def _wrap16(entries):
    """entries: [NI] int -> int16 wrapped layout [128, NI//16]: entry i at
    partition i%16, column i//16, replicated across the 8 groups."""
    s = entries.reshape(-1, 16).T.astype(np.int16)  # [16, NI//16]
    return np.ascontiguousarray(np.tile(s, (8, 1)))


def prep_inputs(nodes, neigh_idx, embed_matrix):
    nodes = np.asarray(nodes)
    neigh_idx = np.asarray(neigh_idx)
    emb = np.ascontiguousarray(np.asarray(embed_matrix, dtype=np.float32))
    idx_full = np.concatenate([neigh_idx, nodes[:, None]], axis=1).astype(
        np.int32
    )  # [N, 33]
    in_maps = []
    for c in range(NCORES):
        slab = idx_full[c * NPR : (c + 1) * NPR]  # [512, 33]
        # entry order i = g*128 + p, g = nb*33 + k  ->  value idx[nb*128+p, k]
        e = slab.reshape(NB, P, KP1).transpose(0, 2, 1).reshape(NI)
        in_maps.append(
            {
                "idx": np.ascontiguousarray(slab),
                "idxw": _wrap16(e),
                "embed": emb,
            }
        )
    return in_maps


def kernel(nodes, neigh_idx, embed_matrix):
    nc = get_nc()
    from concourse.bass_utils import run_bass_kernel_spmd

    in_maps = prep_inputs(nodes, neigh_idx, embed_matrix)
    res = run_bass_kernel_spmd(nc, in_maps, core_ids=list(range(NCORES)))
    out = np.concatenate([res.results[c]["out"] for c in range(NCORES)], axis=0)
    return out.astype(np.float32)

